# revision 4
# baseline (speedup 1.0000x reference)
"""Dice + contrastive loss on 8 Trainium2 NeuronCores — v3 (fp8, tuned).

Same math as v2; scheduling fixes:
  - in12/mask DMA'd in small leading pieces so completion semaphores fire
    early (DMA completion latency ~1.6us/transfer)
  - PE emission keeps A/B Grams (sigmoid-gated) ahead of gt-gated work
  - PSUM evacs split: A/B/E/C on Vector after dm4, D on Scalar after sigp
  - three independent output DMAs so early Grams stream during sigmoid(pred)
"""

import os
import sys

sys.path.insert(0, "/opt/trn_rl_repo")

import numpy as np
import ml_dtypes

import concourse.bass as bass
import concourse.tile as tile
from concourse import bacc, mybir
from concourse.bass_utils import run_bass_kernel_spmd

TAU = 0.1
DICE_SMOOTH = 0.1
WEIGHT = 1.0

NCORES = 8
B = 16
NPIX = 512 * 512
PIX = NPIX // NCORES
P = 128
F = PIX // P
T = 32
S = F // T
NC = B * F                  # 4096

F32 = mybir.dt.float32
BF16 = mybir.dt.bfloat16
F8 = mybir.dt.float8e4
AF = mybir.ActivationFunctionType
ALU = mybir.AluOpType
PM = mybir.MatmulPerfMode

NP_BF16 = ml_dtypes.bfloat16
NP_F8 = ml_dtypes.float8_e4m3

IN12_COLS = [512, 1536, 2048, 2048, 2048]    # s12-col pieces (sum 8192)
MASK_COLS = [1024, 1024, 2048]               # mask-col pieces (sum 4096)
SIGP_COLS = [3072, 1024]                     # sigmoid(pred) pieces


def _build_program():
    nc = bacc.Bacc("TRN2", target_bir_lowering=False, debug=False,
                   num_devices=NCORES)

    d_in12 = nc.dram_tensor("in12", [P, 2 * NC], F8, kind="ExternalInput")
    d_mask = nc.dram_tensor("mask", [P, NC], BF16, kind="ExternalInput")
    d_pred = nc.dram_tensor("pred", [P, NC], F8, kind="ExternalInput")
    d_gt = nc.dram_tensor("gt", [P, NC], F8, kind="ExternalInput")

    d_o1 = nc.dram_tensor("o1", [P, 512], F32, kind="ExternalOutput")   # A|B|C
    d_o2 = nc.dram_tensor("o2", [P, 130], F32, kind="ExternalOutput")   # D|sum_p
    d_oe = nc.dram_tensor("oe", [16, 512], F32, kind="ExternalOutput")  # E

    with tile.TileContext(nc) as tc:
        with tc.tile_pool(name="main", bufs=1) as pool:
            t_in12 = [pool.tile([P, c], F8, name=f"in12_{i}", tag=f"in12_{i}")
                      for i, c in enumerate(IN12_COLS)]
            t_mask = [pool.tile([P, c], BF16, name=f"mask_{i}", tag=f"mask_{i}")
                      for i, c in enumerate(MASK_COLS)]
            t_pred = pool.tile([P, NC], F8, tag="pred")
            t_gt = pool.tile([P, NC], F8, tag="gt")
            s12 = pool.tile([P, 2 * NC], F8, tag="s12")
            t_p = pool.tile([P, NC], F8, tag="p")
            dd = pool.tile([P, NC], BF16, tag="dd")
            dm = pool.tile([P, NC], BF16, tag="dm")
            ones8 = pool.tile([P, 32], F8, tag="ones8")
            sb1 = pool.tile([P, 512], F32, tag="sb1")
            sb2 = pool.tile([P, 130], F32, tag="sb2")
            oe_sb = pool.tile([16, 512], F32, tag="oe_sb")
            with tc.tile_pool(name="psum", bufs=1, space="PSUM") as pp:
                psA = pp.tile([P, 256], F32, tag="psA")
                psB = pp.tile([P, 128], F32, tag="psB")
                psC = pp.tile([P, 128], F32, tag="psC")
                psD = pp.tile([P, 128], F32, tag="psD")
                psE = pp.tile([16, 512], F32, tag="psE")

                # ---- input DMAs (sync FIFO: emission = arrival order) ----
                off = 0
                for i, c in enumerate(IN12_COLS):
                    nc.sync.dma_start(t_in12[i][:], d_in12.ap()[:, off:off + c])
                    off += c
                nc.sync.dma_start(t_mask[0][:], d_mask.ap()[:, 0:1024])
                nc.sync.dma_start(t_mask[1][:], d_mask.ap()[:, 1024:2048])
                nc.sync.dma_start(t_pred[:, 0:3072], d_pred.ap()[:, 0:3072])
                nc.sync.dma_start(t_mask[2][:], d_mask.ap()[:, 2048:4096])
                nc.sync.dma_start(t_pred[:, 3072:4096], d_pred.ap()[:, 3072:4096])
                nc.sync.dma_start(t_gt[:], d_gt.ap())
                nc.vector.memset(ones8[:], 1.0)

                # ---- ACT: sigmoid slabs + sigmoid(pred) in 2 pieces ----
                off = 0
                for i, c in enumerate(IN12_COLS):
                    nc.scalar.activation(s12[:, off:off + c], t_in12[i][:],
                                         AF.Sigmoid)
                    off += c
                off = 0
                for i, c in enumerate(SIGP_COLS):
                    nc.scalar.activation(
                        t_p[:, off:off + c], t_pred[:, off:off + c], AF.Sigmoid,
                        accum_out=sb2[:, 128 + i:129 + i])
                    off += c

                # ---- DVE: d/dm quarters ----
                v12 = s12[:].rearrange("p (t h c) -> p t h c", h=2, c=P)
                vd = dd[:].rearrange("p (t c) -> p t c", c=P)
                vm = dm[:].rearrange("p (t c) -> p t c", c=P)
                mq = [t_mask[0][:], t_mask[1][:],
                      t_mask[2][:, 0:1024], t_mask[2][:, 1024:2048]]
                for q in range(4):
                    sl = slice(q * 8, (q + 1) * 8)
                    nc.vector.tensor_tensor(vd[:, sl], v12[:, sl, 0, :],
                                            v12[:, sl, 1, :], ALU.subtract)
                    nc.vector.tensor_tensor(
                        vm[:, sl], vd[:, sl],
                        mq[q].rearrange("p (t c) -> p t c", c=P), ALU.mult)

                # ---- PE ----
                TP = T // 2

                def ab_pairs(lo, hi):
                    for Tp in range(lo, hi):
                        blk = s12[:, Tp * 512:(Tp + 1) * 512].rearrange(
                            "p (h c) -> p h c", h=2)
                        nc.tensor.matmul(psA[:], blk[:, :, 0:128], blk,
                                         start=(Tp == 0), stop=(Tp == TP - 1),
                                         perf_mode=PM.DoubleRow)
                        nc.tensor.matmul(psB[:], blk[:, :, 128:256],
                                         blk[:, :, 128:256],
                                         start=(Tp == 0), stop=(Tp == TP - 1),
                                         perf_mode=PM.DoubleRow)

                def c_chunks(lo, hi):
                    for t in range(lo, hi):
                        ch = dm[:, t * P:(t + 1) * P]
                        nc.tensor.matmul(psC[:], ch, ch,
                                         start=(t == 0), stop=(t == T - 1))

                def e_colsum(src, m, start, stop):
                    # ones-stationary colsum of src cols [m*1024,(m+1)*1024)
                    nc.tensor.matmul(
                        psE[:], ones8[:].rearrange("p (h c) -> p h c", h=2),
                        src[:, m * 1024:(m + 1) * 1024].rearrange(
                            "p (h c) -> p h c", h=2),
                        start=start, stop=stop, perf_mode=PM.DoubleRow)

                def d_pairs(lo, hi):
                    for Tp in range(lo, hi):
                        lv = t_p[:, Tp * 256:(Tp + 1) * 256].rearrange(
                            "p (h c) -> p h c", h=2)
                        rv = t_gt[:, Tp * 256:(Tp + 1) * 256].rearrange(
                            "p (h c) -> p h c", h=2)
                        nc.tensor.matmul(psD[:], lv, rv,
                                         start=(Tp == 0), stop=(Tp == TP - 1),
                                         perf_mode=PM.DoubleRow)

                ab_pairs(0, 2)
                ab_pairs(2, 4)
                c_chunks(0, 8)
                ab_pairs(4, 8)
                c_chunks(8, 16)
                ab_pairs(8, 12)
                ab_pairs(12, 16)
                c_chunks(16, 24)
                for m in range(4):
                    e_colsum(t_gt, m, start=(m == 0), stop=(m == 3))
                c_chunks(24, 32)
                # sigmoid(pred) waves: D pairs follow each sigp piece
                d_pairs(0, 12)
                d_pairs(12, 16)

                # ---- evac + out ----
                nc.vector.tensor_copy(sb1[:, 0:256], psA[:])
                nc.vector.tensor_copy(sb1[:, 256:384], psB[:])
                nc.vector.tensor_copy(sb1[:, 384:512], psC[:])
                nc.vector.tensor_copy(oe_sb[:], psE[:])
                nc.scalar.copy(sb2[:, 0:128], psD[:])
                nc.sync.dma_start(d_o1.ap(), sb1[:])
                nc.sync.dma_start(d_oe.ap(), oe_sb[:])
                nc.scalar.dma_start(d_o2.ap(), sb2[:])

    nc.compile()
    return nc


_NC_CACHE = None


def _get_program():
    global _NC_CACHE
    if _NC_CACHE is None:
        _NC_CACHE = _build_program()
    return _NC_CACHE


def _shard_inputs(pred_labeled, gt_labeled, input1, input2, mask):
    flat = {
        "pred": np.asarray(pred_labeled, dtype=np.float32).reshape(B, NPIX),
        "gt": np.asarray(gt_labeled, dtype=np.float32).reshape(B, NPIX),
        "in1": np.asarray(input1, dtype=np.float32).reshape(B, NPIX),
        "in2": np.asarray(input2, dtype=np.float32).reshape(B, NPIX),
        "mask": np.asarray(mask, dtype=np.float32).reshape(B, NPIX),
    }

    def pack(a, sl, dt):  # [P, (t s b)]
        return np.ascontiguousarray(
            a[:, sl].reshape(B, P, T, S).transpose(1, 2, 3, 0)
            .reshape(P, NC)).astype(dt)

    in_maps = []
    for k in range(NCORES):
        sl = slice(k * PIX, (k + 1) * PIX)
        i1 = flat["in1"][:, sl].reshape(B, P, T, S).transpose(1, 2, 3, 0)
        i2 = flat["in2"][:, sl].reshape(B, P, T, S).transpose(1, 2, 3, 0)
        in12 = np.stack([i1, i2], axis=2)  # [P, T, 2, S, B]
        in_maps.append({
            "in12": np.ascontiguousarray(in12.reshape(P, 2 * NC)).astype(NP_F8),
            "mask": pack(flat["mask"], sl, NP_BF16),
            "pred": pack(flat["pred"], sl, NP_F8),
            "gt": pack(flat["gt"], sl, NP_F8),
        })
    return in_maps


def _block_diag_sum(gmat):
    g = gmat.reshape(S, B, S, B)
    return np.einsum("sbsc->bc", g)


def _combine(results):
    sum_pg = sum_pg_den = 0.0
    g1 = np.zeros((B, B), np.float64)
    cr = np.zeros((B, B), np.float64)
    g2 = np.zeros((B, B), np.float64)
    pc = np.zeros((B, B), np.float64)
    for r in results:
        o1 = r["o1"].astype(np.float64)
        o2 = r["o2"].astype(np.float64)
        g1 += _block_diag_sum(o1[:, 0:128])
        cr += _block_diag_sum(o1[:, 128:256])
        g2 += _block_diag_sum(o1[:, 256:384])
        pc += _block_diag_sum(o1[:, 384:512])
        sum_pg += np.trace(o2[:, 0:128])
        sum_pg_den += o2[:, 128:130].sum()                 # sum_p
        sum_pg_den += r["oe"].astype(np.float64)[0].sum()  # sum_g

    dice = 1.0 - (2.0 * sum_pg + DICE_SMOOTH) / (sum_pg_den + DICE_SMOOTH)

    n = float(NPIX)
    sq1 = np.diag(g1) / n
    sq2 = np.diag(g2) / n
    cross = cr / n
    pos_mse = np.diag(pc) / n

    sim_pos = np.exp(-pos_mse / TAU)
    mse = sq1[:, None] + sq2[None, :] - 2.0 * cross
    sim = np.exp(-mse / TAU)
    sim_neg = (sim * (1.0 - np.eye(B))).sum(axis=1)
    loss_c = float(np.mean(-np.log(sim_pos / (sim_pos + sim_neg))))
    total = dice + WEIGHT * loss_c
    return (np.float32(total), np.float32(dice), 0.0, np.float32(loss_c))


def kernel(pred_labeled, gt_labeled, input1, input2, mask):
    nc = _get_program()
    in_maps = _shard_inputs(pred_labeled, gt_labeled, input1, input2, mask)
    res = run_bass_kernel_spmd(nc, in_maps, core_ids=list(range(NCORES)),
                               trace=bool(int(os.environ.get("KERNEL_TRACE", "0"))))
    out = _combine(res.results)
    if res.exec_time_ns is not None:
        print(f"HW exec time: {res.exec_time_ns} ns")
    return out
